# revision 33
# baseline (speedup 1.0000x reference)
"""Sparse diag-masked multi-head attention layer on 8 trn2 cores.

Sharding: core = b*4 + g  (b in 0..1 batches, g in 0..3 head-groups).
Each core computes heads 4g..4g+3 of batch b. Head-group g has band
offset off = 2**g: visible(q, s) <=> s >= q + off  OR  s == L-1.

v2 layout (all matmul operands bf16, PSUM f32):
  qT, kT  [e=256, L] bf16   (2 sbuf tiles of [128, L])   e on partitions
  v_pack  16 x [128 s, 4 h, 66] bf16  (cols 0:64 v, col 64 ones)
  scoresT [s 128, q ev] psum = kT_chunk.T @ qT_chunk  (contract e=64)
  P = exp(0.125*scoresT) -> sbuf bf16, band-masked via (iota<=T[p])*P
      (iota + thresholds in fp16 so the DVE mask runs in 4x mode)
  num/den: psum_o[h] [65, ev] += v_aug.T @ P   (contract s)
  attnT   [e, q] bf16 = num * bcast(1/den)  (DVE reads psum directly)
  outT    [n, q] psum = woT_chunk.T @ attnT_chunk; DVE copy -> bf16 st
  out-projection of chunk j is interleaved into chunk j+1's score loop
  to keep the PE fed while the normalize chain drains.

Host: out[b] = sum_g outT(b,g).T + bv @ Wo.T + bo  (f32 accumulate)
"""
import sys

sys.path.insert(0, "/opt/trn_rl_repo")

import numpy as np

import concourse.bacc as bacc
import concourse.bass as bass
import concourse.mybir as mybir
import concourse.tile as tile

P = 128
L = 2048
D = 1024
EPC = 256  # head-dims per core (4 heads x 64)
EC = 2  # e-chunks of 128
HPC = 4  # heads per core
NQ = 512  # q-chunk width
NJ = L // NQ  # 4
NSB = L // P  # 16 s-blocks
NDC = D // P  # 8 d-chunks
SCALE = 0.125  # 1/sqrt(64)

F32 = mybir.dt.float32
F32R = mybir.dt.float32r
BF16 = mybir.dt.bfloat16
F16 = mybir.dt.float16
X_DT = BF16  # dtype of streamed activations + qkv weights (DMA halving)


def r(ap):
    return ap.bitcast(F32R)


def build_nc():
    nc = bacc.Bacc("TRN2", target_bir_lowering=False, debug=False)

    xq = nc.dram_tensor("xqT", [D, L], X_DT, kind="ExternalInput")
    xk = nc.dram_tensor("xkT", [D, L], X_DT, kind="ExternalInput")
    xv = nc.dram_tensor("xvT", [D, L], X_DT, kind="ExternalInput")
    wq = nc.dram_tensor("wqT", [D, EPC], X_DT, kind="ExternalInput")
    wk = nc.dram_tensor("wkT", [D, EPC], X_DT, kind="ExternalInput")
    wv = nc.dram_tensor("wvT", [D, EPC], X_DT, kind="ExternalInput")
    wo = nc.dram_tensor("woT", [EPC, D], F32, kind="ExternalInput")
    bqk = nc.dram_tensor("bqk", [2, EC, P], F32, kind="ExternalInput")
    ramps = nc.dram_tensor("ramps", [2, P], F32, kind="ExternalInput")
    out = nc.dram_tensor("outT", [D, L], BF16, kind="ExternalOutput")

    with tile.TileContext(nc) as tc:
        with (
            tc.tile_pool(name="consts", bufs=1) as consts,
            tc.tile_pool(name="acts", bufs=1) as acts,
        ):
            # ---- constants / weights
            wq_sb = consts.tile([P, NDC, EPC], X_DT, tag="wq")
            wk_sb = consts.tile([P, NDC, EPC], X_DT, tag="wk")
            wv_sb = consts.tile([P, NDC, EPC], X_DT, tag="wv")
            wv_r = wv.rearrange("(dc p) e -> p dc e", p=P)
            nc.sync.dma_start(wv_sb[:, 0:4, :], wv_r[:, 0:4, :])
            wo_sb = consts.tile([P, EC, D], F32R, tag="wo")
            wo_raw = consts.tile([P, EC, D], F32, tag="wo_raw")
            bqk_sb = consts.tile([P, 2, EC], F32, tag="bqk")
            ramps_sb = consts.tile([P, 2], F32, tag="ramps")

            ones_col = consts.tile([P, 1], F32, tag="ones_col")
            nc.vector.memset(ones_col[:], 1.0)
            c0i = consts.tile([P, NQ], mybir.dt.int32, tag="c0i")
            nc.gpsimd.iota(c0i[:], pattern=[[1, NQ]], base=0, channel_multiplier=0)
            c0h = consts.tile([P, NQ], F32, tag="c0h")
            nc.vector.tensor_copy(c0h[:], c0i[:])

            # ---- persistent activations
            qT = [acts.tile([P, L], F32R, tag=f"qT{c}", name=f"qT{c}") for c in range(EC)]
            kT = [acts.tile([P, L], F32R, tag=f"kT{c}", name=f"kT{c}") for c in range(EC)]
            aT = [acts.tile([P, L], F32R, tag=f"aT{c}", name=f"aT{c}") for c in range(EC)]
            vpk = [acts.tile([P, HPC, 66], F32R, tag=f"v{sb}", name=f"v{sb}") for sb in range(NSB)]

            # ================= phase 1: projections =================
            with (
                tc.tile_pool(name="xs", bufs=4) as xpool,
                tc.tile_pool(name="pjp", bufs=4, space=bass.MemorySpace.PSUM) as pjp,
            ):
                # v projection first (attention needs all of vpk; also the
                # heaviest DMA). Quad-grouped x tiles: one DMA feeds 4 s-blocks.
                xv_r = xv.rearrange("(dc p) l -> p dc l", p=P)
                for qd in range(NSB // 4 - 1, -1, -1):
                    xt = xpool.tile([P, NDC, 4 * P], X_DT, tag="xv", name="xv", bufs=3)
                    if qd == NSB // 4 - 1:
                        for dc in range(NDC):
                            nc.sync.dma_start(
                                xt[:, dc, :],
                                xv_r[:, dc, qd * 4 * P : (qd + 1) * 4 * P],
                            )
                            if dc == 3:
                                nc.sync.dma_start(wv_sb[:, 4:8, :], wv_r[:, 4:8, :])
                    else:
                        nc.sync.dma_start(
                            xt[:], xv_r[:, :, qd * 4 * P : (qd + 1) * 4 * P]
                        )
                    for si in range(3, -1, -1):
                        sb = qd * 4 + si
                        psv = pjp.tile([P, EPC], F32, tag="vps", name="vps", bufs=2)
                        for dc in range(NDC):
                            nc.tensor.matmul(
                                psv[:],
                                xt[:, dc, si * P : (si + 1) * P],
                                wv_sb[:, dc, :],
                                start=(dc == 0),
                                stop=(dc == NDC - 1),
                            )
                        nc.scalar.copy(
                            vpk[sb][:, :, 0:64],
                            psv[:].rearrange("p (h e) -> p h e", e=64),
                        )
                        nc.vector.tensor_copy(
                            vpk[sb][:, :, 64:65],
                            ones_col[:, None, :].broadcast_to([P, HPC, 1]),
                        )

                nc.sync.dma_start(bqk_sb[:], bqk.rearrange("t c p -> p t c"))
                nc.sync.dma_start(ramps_sb[:], ramps.rearrange("t p -> p t"))

                for ti, (xdram, wsb) in enumerate(((xk, wk_sb), (xq, wq_sb))):
                    dst = (kT, qT)[ti]
                    bias_t = 1 - ti  # bqk row 0 = bq, row 1 = bk
                    wdram = (wk, wq)[ti]
                    nc.sync.dma_start(wsb[:], wdram.rearrange("(dc p) e -> p dc e", p=P))
                    for lp in ((1, 0) if ti == 0 else (0, 1)):  # k: tail first
                        ps = [
                            [pjp.tile([P, NQ], F32, tag="pj", name="pj", bufs=6) for _ in range(2)]
                            for _ in range(EC)
                        ]
                        for dc in range(NDC):
                            xt = xpool.tile([P, 2 * NQ], X_DT, tag=f"x{ti}", name="xt", bufs=8)
                            nc.sync.dma_start(
                                xt[:],
                                xdram[
                                    dc * P : (dc + 1) * P,
                                    lp * 2 * NQ : (lp + 1) * 2 * NQ,
                                ],
                            )
                            for ec in range(EC):
                                for l2 in range(2):
                                    nc.tensor.matmul(
                                        ps[ec][l2][:],
                                        wsb[:, dc, ec * P : (ec + 1) * P],
                                        xt[:, l2 * NQ : (l2 + 1) * NQ],
                                        start=(dc == 0),
                                        stop=(dc == NDC - 1),
                                    )
                        for l2 in range(2):
                            for ec in range(EC):
                                lc = lp * 2 + l2
                                # split each psum pair across Act/DVE so the
                                # last adds (gating the pool transition) drain
                                # in parallel; q's first chunk fully on Act
                                on_act = ec == 0 or (ti == 1 and lp == 0 and l2 == 0)
                                if on_act:
                                    nc.scalar.activation(
                                        dst[ec][:, lc * NQ : (lc + 1) * NQ],
                                        ps[ec][l2][:],
                                        mybir.ActivationFunctionType.Identity,
                                        bias=bqk_sb[:, bias_t, ec : ec + 1],
                                        scale=1.0,
                                    )
                                else:
                                    nc.vector.tensor_scalar_add(
                                        dst[ec][:, lc * NQ : (lc + 1) * NQ],
                                        ps[ec][l2][:],
                                        bqk_sb[:, bias_t, ec : ec + 1],
                                    )

                nc.sync.dma_start(wo_raw[:], wo.rearrange("(c p) n -> p c n", p=P))
                nc.gpsimd.tensor_copy(wo_sb[:], wo_raw[:])

            # ================= phase 2: attention =================
            with (
                tc.tile_pool(name="scp", bufs=2, space=bass.MemorySpace.PSUM) as scp,
                tc.tile_pool(name="avp", bufs=4, space=bass.MemorySpace.PSUM) as avp,
                tc.tile_pool(name="pp", bufs=7) as pp,
                tc.tile_pool(name="tp", bufs=4) as tp,
                tc.tile_pool(name="rdp", bufs=4) as rdp,
                tc.tile_pool(name="rbp", bufs=4) as rbp,
                tc.tile_pool(name="osb", bufs=4) as osb,
            ):
                def emit_op_group(jo, np2, st_on_act=False):
                    # out-projection columns [np2*256, np2*256+256) for q-chunk jo
                    qo = jo * NQ
                    ops = scp.tile([P, 2, NQ], F32, tag="sc", name="op")
                    for i in range(2):
                        n = np2 * 2 + i
                        for c in range(EC):
                            nc.tensor.matmul(
                                ops[:, i, :],
                                r(wo_sb[:, c, n * P : (n + 1) * P]),
                                r(aT[c][:, qo : qo + NQ]),
                                start=(c == 0),
                                stop=(c == EC - 1),
                            )
                    st = osb.tile([P, 2, NQ], BF16, tag="ost", name="ost")
                    if st_on_act:
                        nc.scalar.copy(st[:], ops[:])
                    else:
                        nc.vector.tensor_copy(st[:], ops[:])
                    nc.sync.dma_start(
                        out[np2 * 2 * P : (np2 + 1) * 2 * P, qo : qo + NQ].rearrange(
                            "(i p) q -> p i q", i=2
                        ),
                        st[:],
                    )

                for j in range(NJ):
                    q0 = j * NQ
                    js_lo = 4 * j
                    av = [avp.tile([65, NQ], F32, tag="av", name="av") for _ in range(HPC)]
                    # js descending: accumulation group starts at js=15 with a
                    # full-width MM; diagonal tiles then accumulate a prefix.
                    for js in range(NSB - 1, js_lo - 1, -1):
                        s0 = js * P
                        k = js - js_lo
                        masked = k <= 4
                        # visible q-prefix (min 256: f32r matmuls below N=256 run at 1/4 rate)
                        ev = min(NQ, max(2 * P, (k + 1) * P))
                        tt = None
                        for pr in range(2):  # head pairs (0,1), (2,3)
                            sp = scp.tile([P, 2, NQ], F32, tag="sc", name="sc")
                            for i in range(2):
                                nc.tensor.matmul(
                                    sp[:, i, 0:ev],
                                    r(kT[pr][i * 64 : i * 64 + 64, s0 : s0 + P]),
                                    r(qT[pr][i * 64 : i * 64 + 64, q0 : q0 + ev]),
                                    start=True,
                                    stop=True,
                                )
                            pt = pp.tile([P, 2, NQ], F32R, tag="p", name="p")
                            nc.scalar.activation(
                                pt[:, :, 0:ev],
                                sp[:, :, 0:ev],
                                mybir.ActivationFunctionType.Exp,
                                scale=SCALE,
                            )
                            if masked:
                                if tt is None:
                                    tt = tp.tile([P, 1], F32, tag="t", name="t")
                                    ramp = ramps_sb[:, 1:2] if js == NSB - 1 else ramps_sb[:, 0:1]
                                    nc.vector.tensor_scalar_add(
                                        tt[:], ramp, float(s0 - q0)
                                    )
                                m0 = max(0, k * P - 8)
                                nc.vector.scalar_tensor_tensor(
                                    pt[:, :, m0:ev],
                                    c0h[:, None, m0:ev].broadcast_to([P, 2, ev - m0]),
                                    tt[:],
                                    pt[:, :, m0:ev],
                                    op0=mybir.AluOpType.is_le,
                                    op1=mybir.AluOpType.mult,
                                )
                            for i in range(2):
                                h = 2 * pr + i
                                nc.tensor.matmul(
                                    av[h][:, 0:ev],
                                    r(vpk[js][:, h, 0:65]),
                                    r(pt[:, i, 0:ev]),
                                    start=(js == NSB - 1),
                                    stop=(js == js_lo),
                                )
                    # out projection of the previous j-chunk, emitted
                    # BEFORE this j's normalize: its DVE st copies drain ahead
                    # of the normalize chain so score-psum slots free early.
                    # The last j normalizes first (no next-j work hides it) and
                    # its predecessor's sts go to the idle Act engine instead.
                    if 0 < j < NJ - 1:
                        for np2 in range(4 if j == 1 else 2):
                            emit_op_group(j - 1, np2)
                    if j == NJ - 1:
                        for np2 in range(4):
                            emit_op_group(j - 1, np2, st_on_act=True)
                        # deferred ops(1) groups: PE filler while normalize(3)
                        # drains on DVE/Pool (nothing else remains at the tail)
                        for np2 in (2, 3):
                            emit_op_group(1, np2, st_on_act=True)
                    for h in range(HPC):
                        ec, r0 = h // 2, (h % 2) * 64
                        rd = rdp.tile([1, NQ], F32, tag="rd")
                        nc.vector.reciprocal(rd[:], av[h][64:65, :])
                        rb = rbp.tile([64, NQ], F32, tag="rb")
                        nc.gpsimd.partition_broadcast(rb[:], rd[:], channels=64)
                        nc.vector.tensor_mul(
                            aT[ec][r0 : r0 + 64, q0 : q0 + NQ],
                            av[h][0:64, :],
                            rb[:],
                        )
                # final j: fused st tile, quarter DMAs fire per copy
                stf = osb.tile([P, NDC, NQ], BF16, tag="stf", name="stf", bufs=1)
                qo = (NJ - 1) * NQ
                for np2 in range(4):
                    ops = scp.tile([P, 2, NQ], F32, tag="sc", name="op")
                    for i in range(2):
                        n = np2 * 2 + i
                        for c in range(EC):
                            nc.tensor.matmul(
                                ops[:, i, :],
                                r(wo_sb[:, c, n * P : (n + 1) * P]),
                                r(aT[c][:, qo : qo + NQ]),
                                start=(c == 0),
                                stop=(c == EC - 1),
                            )
                    nc.scalar.copy(stf[:, np2 * 2 : np2 * 2 + 1, :], ops[:, 0:1, :])
                    nc.vector.tensor_copy(stf[:, np2 * 2 + 1 : np2 * 2 + 2, :], ops[:, 1:2, :])
                    nc.sync.dma_start(
                        out[np2 * 2 * P : (np2 + 1) * 2 * P, qo : qo + NQ].rearrange(
                            "(i p) q -> p i q", i=2
                        ),
                        stf[:, np2 * 2 : np2 * 2 + 2, :],
                    )

    nc.compile()
    return nc


def make_in_maps(queries, keys, values, Wq, bq, Wk, bk, Wv, bv, Wo, bo):
    """Build per-core input maps. core = b*4 + g."""
    f32 = np.float32
    import ml_dtypes
    x_dt = ml_dtypes.bfloat16
    in_maps = []
    for core in range(8):
        b, g = core // 4, core % 4
        cols = slice(g * EPC, (g + 1) * EPC)
        off = 2 ** g
        ramp = (np.arange(P) - off).astype(f32)
        ramp_last = ramp.copy()
        ramp_last[P - 1] = 1e9  # s == L-1 always visible
        in_maps.append(
            {
                "xqT": np.ascontiguousarray(queries[b].T).astype(x_dt),
                "xkT": np.ascontiguousarray(keys[b].T).astype(x_dt),
                "xvT": np.ascontiguousarray(values[b].T).astype(x_dt),
                "wqT": np.ascontiguousarray(Wq[cols, :].T).astype(x_dt),
                "wkT": np.ascontiguousarray(Wk[cols, :].T).astype(x_dt),
                "wvT": np.ascontiguousarray(Wv[cols, :].T).astype(x_dt),
                "woT": np.ascontiguousarray(Wo[:, cols].T, dtype=f32),
                "bqk": np.stack(
                    [bq[cols].reshape(EC, P), bk[cols].reshape(EC, P)]
                ).astype(f32),
                "ramps": np.stack([ramp, ramp_last]),
            }
        )
    return in_maps


def gather_outputs(results, Wo, bv, bo):
    """results: list of 8 dicts with 'outT' [D, L] bf16. Returns [2, L, D] f32."""
    host_bias = (Wo.astype(np.float64) @ bv.astype(np.float64) + bo).astype(
        np.float32
    )
    out = np.zeros((2, L, D), np.float32)
    for b in range(2):
        acc = np.zeros((D, L), np.float32)
        for g in range(4):
            acc += np.asarray(results[b * 4 + g]["outT"]).astype(np.float32)
        out[b] = acc.T + host_bias[None, :]
    return out




# ======================= host entry point =======================
_NC_CACHE = None


def kernel(queries, keys, values, Wq, bq, Wk, bk, Wv, bv, Wo, bo):
    """Full-input entry: shards across 8 NeuronCores, returns [2, 2048, 1024]."""
    global _NC_CACHE
    from concourse.bass_utils import run_bass_kernel_spmd

    args = [np.asarray(a) for a in (queries, keys, values, Wq, bq, Wk, bk, Wv, bv, Wo, bo)]
    queries, keys, values, Wq, bq, Wk, bk, Wv, bv, Wo, bo = args
    if _NC_CACHE is None:
        _NC_CACHE = build_nc()
    in_maps = make_in_maps(queries, keys, values, Wq, bq, Wk, bk, Wv, bv, Wo, bo)
    res = run_bass_kernel_spmd(_NC_CACHE, in_maps, list(range(8)))
    return gather_outputs(res.results, Wo, bv, bo)


# revision 35
# speedup vs baseline: 1.0248x; 1.0248x over previous
"""Sparse diag-masked multi-head attention layer on 8 trn2 cores.

Sharding: core = b*4 + g  (b in 0..1 batches, g in 0..3 head-groups).
Each core computes heads 4g..4g+3 of batch b. Head-group g has band
offset off = 2**g: visible(q, s) <=> s >= q + off  OR  s == L-1.

v2 layout (all matmul operands bf16, PSUM f32):
  qT, kT  [e=256, L] bf16   (2 sbuf tiles of [128, L])   e on partitions
  v_pack  16 x [128 s, 4 h, 66] bf16  (cols 0:64 v, col 64 ones)
  scoresT [s 128, q ev] psum = kT_chunk.T @ qT_chunk  (contract e=64)
  P = exp(0.125*scoresT) -> sbuf bf16, band-masked via (iota<=T[p])*P
      (iota + thresholds in fp16 so the DVE mask runs in 4x mode)
  num/den: psum_o[h] [65, ev] += v_aug.T @ P   (contract s)
  attnT   [e, q] bf16 = num * bcast(1/den)  (DVE reads psum directly)
  outT    [n, q] psum = woT_chunk.T @ attnT_chunk; DVE copy -> bf16 st
  out-projection of chunk j is interleaved into chunk j+1's score loop
  to keep the PE fed while the normalize chain drains.

Host: out[b] = sum_g outT(b,g).T + bv @ Wo.T + bo  (f32 accumulate)
"""
import sys

sys.path.insert(0, "/opt/trn_rl_repo")

import numpy as np

import concourse.bacc as bacc
import concourse.bass as bass
import concourse.mybir as mybir
import concourse.tile as tile

P = 128
L = 2048
D = 1024
EPC = 256  # head-dims per core (4 heads x 64)
EC = 2  # e-chunks of 128
HPC = 4  # heads per core
NQ = 512  # q-chunk width
NJ = L // NQ  # 4
NSB = L // P  # 16 s-blocks
NDC = D // P  # 8 d-chunks
SCALE = 0.125  # 1/sqrt(64)

F32 = mybir.dt.float32
F32R = mybir.dt.float32r
BF16 = mybir.dt.bfloat16
F16 = mybir.dt.float16
X_DT = BF16  # dtype of streamed activations + qkv weights (DMA halving)


def r(ap):
    return ap.bitcast(F32R)


def build_nc():
    nc = bacc.Bacc("TRN2", target_bir_lowering=False, debug=False)

    xq = nc.dram_tensor("xqT", [D, L], X_DT, kind="ExternalInput")
    xk = nc.dram_tensor("xkT", [D, L], X_DT, kind="ExternalInput")
    xv = nc.dram_tensor("xvT", [D, L], X_DT, kind="ExternalInput")
    wq = nc.dram_tensor("wqT", [D, EPC], X_DT, kind="ExternalInput")
    wk = nc.dram_tensor("wkT", [D, EPC], X_DT, kind="ExternalInput")
    wv = nc.dram_tensor("wvT", [D, EPC], X_DT, kind="ExternalInput")
    wo = nc.dram_tensor("woT", [EPC, D], F32, kind="ExternalInput")
    bqk = nc.dram_tensor("bqk", [2, EC, P], F32, kind="ExternalInput")
    ramps = nc.dram_tensor("ramps", [2, P], F32, kind="ExternalInput")
    out = nc.dram_tensor("outT", [D, L], BF16, kind="ExternalOutput")

    with tile.TileContext(nc) as tc:
        with (
            tc.tile_pool(name="consts", bufs=1) as consts,
            tc.tile_pool(name="acts", bufs=1) as acts,
        ):
            # ---- constants / weights
            wq_sb = consts.tile([P, NDC, EPC], X_DT, tag="wq")
            wk_sb = consts.tile([P, NDC, EPC], X_DT, tag="wk")
            wv_sb = consts.tile([P, NDC, EPC], X_DT, tag="wv")
            wv_r = wv.rearrange("(dc p) e -> p dc e", p=P)
            nc.sync.dma_start(wv_sb[:, 0:4, :], wv_r[:, 0:4, :])
            wo_sb = consts.tile([P, EC, D], F32R, tag="wo")
            wo_raw = consts.tile([P, EC, D], F32, tag="wo_raw")
            bqk_sb = consts.tile([P, 2, EC], F32, tag="bqk")
            ramps_sb = consts.tile([P, 2], F32, tag="ramps")

            ones_col = consts.tile([P, 1], F32, tag="ones_col")
            nc.vector.memset(ones_col[:], 1.0)
            c0i = consts.tile([P, NQ], mybir.dt.int32, tag="c0i")
            nc.gpsimd.iota(c0i[:], pattern=[[1, NQ]], base=0, channel_multiplier=0)
            c0h = consts.tile([P, NQ], F32, tag="c0h")
            nc.vector.tensor_copy(c0h[:], c0i[:])

            # ---- persistent activations
            qT = [acts.tile([P, L], F32R, tag=f"qT{c}", name=f"qT{c}") for c in range(EC)]
            kT = [acts.tile([P, L], F32R, tag=f"kT{c}", name=f"kT{c}") for c in range(EC)]
            aT = [acts.tile([P, L], F32R, tag=f"aT{c}", name=f"aT{c}") for c in range(EC)]
            vpk = [acts.tile([P, HPC, 66], F32R, tag=f"v{sb}", name=f"v{sb}") for sb in range(NSB)]

            # ================= phase 1: projections =================
            with (
                tc.tile_pool(name="xs", bufs=4) as xpool,
                tc.tile_pool(name="pjp", bufs=4, space=bass.MemorySpace.PSUM) as pjp,
            ):
                # v projection first (attention needs all of vpk; also the
                # heaviest DMA). Quad-grouped x tiles: one DMA feeds 4 s-blocks.
                xv_r = xv.rearrange("(dc p) l -> p dc l", p=P)
                for qd in range(NSB // 4 - 1, -1, -1):
                    xt = xpool.tile([P, NDC, 4 * P], X_DT, tag="xv", name="xv", bufs=3)
                    if qd == NSB // 4 - 1:
                        for dc in range(NDC):
                            nc.sync.dma_start(
                                xt[:, dc, :],
                                xv_r[:, dc, qd * 4 * P : (qd + 1) * 4 * P],
                            )
                            if dc == 3:
                                nc.sync.dma_start(wv_sb[:, 4:8, :], wv_r[:, 4:8, :])
                    else:
                        nc.sync.dma_start(
                            xt[:], xv_r[:, :, qd * 4 * P : (qd + 1) * 4 * P]
                        )
                    for si in range(3, -1, -1):
                        sb = qd * 4 + si
                        psv = pjp.tile([P, EPC], F32, tag="vps", name="vps", bufs=2)
                        for dc in range(NDC):
                            nc.tensor.matmul(
                                psv[:],
                                xt[:, dc, si * P : (si + 1) * P],
                                wv_sb[:, dc, :],
                                start=(dc == 0),
                                stop=(dc == NDC - 1),
                            )
                        nc.scalar.copy(
                            vpk[sb][:, :, 0:64],
                            psv[:].rearrange("p (h e) -> p h e", e=64),
                        )
                        nc.vector.tensor_copy(
                            vpk[sb][:, :, 64:65],
                            ones_col[:, None, :].broadcast_to([P, HPC, 1]),
                        )

                nc.sync.dma_start(bqk_sb[:], bqk.rearrange("t c p -> p t c"))
                nc.sync.dma_start(ramps_sb[:], ramps.rearrange("t p -> p t"))

                for ti, (xdram, wsb) in enumerate(((xk, wk_sb), (xq, wq_sb))):
                    dst = (kT, qT)[ti]
                    bias_t = 1 - ti  # bqk row 0 = bq, row 1 = bk
                    wdram = (wk, wq)[ti]
                    nc.sync.dma_start(wsb[:], wdram.rearrange("(dc p) e -> p dc e", p=P))
                    for lp in ((1, 0) if ti == 0 else (0, 1)):  # k: tail first
                        ps = [
                            [pjp.tile([P, NQ], F32, tag="pj", name="pj", bufs=6) for _ in range(2)]
                            for _ in range(EC)
                        ]
                        for dc in range(NDC):
                            xt = xpool.tile([P, 2 * NQ], X_DT, tag=f"x{ti}", name="xt", bufs=8)
                            nc.sync.dma_start(
                                xt[:],
                                xdram[
                                    dc * P : (dc + 1) * P,
                                    lp * 2 * NQ : (lp + 1) * 2 * NQ,
                                ],
                            )
                            for ec in range(EC):
                                for l2 in range(2):
                                    nc.tensor.matmul(
                                        ps[ec][l2][:],
                                        wsb[:, dc, ec * P : (ec + 1) * P],
                                        xt[:, l2 * NQ : (l2 + 1) * NQ],
                                        start=(dc == 0),
                                        stop=(dc == NDC - 1),
                                    )
                        for l2 in range(2):
                            for ec in range(EC):
                                lc = lp * 2 + l2
                                # split each psum pair across Act/DVE so the
                                # last adds (gating the pool transition) drain
                                # in parallel; q's first chunk fully on Act
                                on_act = ec == 0 or (ti == 1 and lp == 0 and l2 == 0)
                                if on_act:
                                    nc.scalar.activation(
                                        dst[ec][:, lc * NQ : (lc + 1) * NQ],
                                        ps[ec][l2][:],
                                        mybir.ActivationFunctionType.Identity,
                                        bias=bqk_sb[:, bias_t, ec : ec + 1],
                                        scale=1.0,
                                    )
                                else:
                                    nc.vector.tensor_scalar_add(
                                        dst[ec][:, lc * NQ : (lc + 1) * NQ],
                                        ps[ec][l2][:],
                                        bqk_sb[:, bias_t, ec : ec + 1],
                                    )

                nc.sync.dma_start(wo_raw[:], wo.rearrange("(c p) n -> p c n", p=P))
                nc.gpsimd.tensor_copy(wo_sb[:], wo_raw[:])

            # ================= phase 2: attention =================
            with (
                tc.tile_pool(name="scp", bufs=2, space=bass.MemorySpace.PSUM) as scp,
                tc.tile_pool(name="avp", bufs=4, space=bass.MemorySpace.PSUM) as avp,
                tc.tile_pool(name="pp", bufs=7) as pp,
                tc.tile_pool(name="tp", bufs=4) as tp,
                tc.tile_pool(name="rdp", bufs=4) as rdp,
                tc.tile_pool(name="rbp", bufs=4) as rbp,
                tc.tile_pool(name="osb", bufs=4) as osb,
            ):
                def emit_op_group(jo, np2, st_on_act=False):
                    # out-projection columns [np2*256, np2*256+256) for q-chunk jo
                    qo = jo * NQ
                    ops = scp.tile([P, 2, NQ], F32, tag="sc", name="op")
                    for i in range(2):
                        n = np2 * 2 + i
                        for c in range(EC):
                            nc.tensor.matmul(
                                ops[:, i, :],
                                r(wo_sb[:, c, n * P : (n + 1) * P]),
                                r(aT[c][:, qo : qo + NQ]),
                                start=(c == 0),
                                stop=(c == EC - 1),
                            )
                    st = osb.tile([P, 2, NQ], BF16, tag="ost", name="ost")
                    if st_on_act:
                        nc.scalar.copy(st[:], ops[:])
                    else:
                        nc.vector.tensor_copy(st[:], ops[:])
                    nc.sync.dma_start(
                        out[np2 * 2 * P : (np2 + 1) * 2 * P, qo : qo + NQ].rearrange(
                            "(i p) q -> p i q", i=2
                        ),
                        st[:],
                    )

                for j in range(NJ):
                    q0 = j * NQ
                    js_lo = 4 * j
                    av = [avp.tile([65, NQ], F32, tag="av", name="av") for _ in range(HPC)]
                    # js descending: accumulation group starts at js=15 with a
                    # full-width MM; diagonal tiles then accumulate a prefix.
                    for js in range(NSB - 1, js_lo - 1, -1):
                        s0 = js * P
                        k = js - js_lo
                        masked = k <= 4
                        # visible q-prefix (min 256: f32r matmuls below N=256 run at 1/4 rate)
                        ev = min(NQ, max(2 * P, (k + 1) * P))
                        # exp only the truly visible prefix; for k=0 the mask
                        # zeroes cols 128:256 (stale-but-finite pool content)
                        ew = min(ev, (k + 1) * P)
                        tt = None
                        for pr in range(2):  # head pairs (0,1), (2,3)
                            sp = scp.tile([P, 2, NQ], F32, tag="sc", name="sc")
                            for i in range(2):
                                nc.tensor.matmul(
                                    sp[:, i, 0:ev],
                                    r(kT[pr][i * 64 : i * 64 + 64, s0 : s0 + P]),
                                    r(qT[pr][i * 64 : i * 64 + 64, q0 : q0 + ev]),
                                    start=True,
                                    stop=True,
                                )
                            pt = pp.tile([P, 2, NQ], F32R, tag="p", name="p")
                            nc.scalar.activation(
                                pt[:, :, 0:ew],
                                sp[:, :, 0:ew],
                                mybir.ActivationFunctionType.Exp,
                                scale=SCALE,
                            )
                            if masked:
                                if tt is None:
                                    tt = tp.tile([P, 1], F32, tag="t", name="t")
                                    ramp = ramps_sb[:, 1:2] if js == NSB - 1 else ramps_sb[:, 0:1]
                                    nc.vector.tensor_scalar_add(
                                        tt[:], ramp, float(s0 - q0)
                                    )
                                m0 = max(0, k * P - 8)
                                nc.vector.scalar_tensor_tensor(
                                    pt[:, :, m0:ev],
                                    c0h[:, None, m0:ev].broadcast_to([P, 2, ev - m0]),
                                    tt[:],
                                    pt[:, :, m0:ev],
                                    op0=mybir.AluOpType.is_le,
                                    op1=mybir.AluOpType.mult,
                                )
                            for i in range(2):
                                h = 2 * pr + i
                                nc.tensor.matmul(
                                    av[h][:, 0:ev],
                                    r(vpk[js][:, h, 0:65]),
                                    r(pt[:, i, 0:ev]),
                                    start=(js == NSB - 1),
                                    stop=(js == js_lo),
                                )
                    # out projection of the previous j-chunk, emitted
                    # BEFORE this j's normalize: its DVE st copies drain ahead
                    # of the normalize chain so score-psum slots free early.
                    # The last j normalizes first (no next-j work hides it) and
                    # its predecessor's sts go to the idle Act engine instead.
                    if 0 < j < NJ - 1:
                        for np2 in range(4 if j == 1 else 2):
                            emit_op_group(j - 1, np2)
                    if j == NJ - 1:
                        for np2 in range(4):
                            emit_op_group(j - 1, np2, st_on_act=True)
                        # deferred ops(1) groups: PE filler while normalize(3)
                        # drains on DVE/Pool (nothing else remains at the tail)
                        for np2 in (2, 3):
                            emit_op_group(1, np2, st_on_act=True)
                    # all recips first: DVE runs them back-to-back while
                    # the Pool broadcasts pipeline behind them; muls last
                    rds, rbs = [], []
                    for h in range(HPC):
                        rd = rdp.tile([1, NQ], F32, tag="rd", bufs=4)
                        nc.vector.reciprocal(rd[:], av[h][64:65, :])
                        rds.append(rd)
                    for h in range(HPC):
                        rb = rbp.tile([64, NQ], F32, tag="rb", bufs=4)
                        nc.gpsimd.partition_broadcast(rb[:], rds[h][:], channels=64)
                        rbs.append(rb)
                    for h in range(HPC):
                        ec, r0 = h // 2, (h % 2) * 64
                        nc.vector.tensor_mul(
                            aT[ec][r0 : r0 + 64, q0 : q0 + NQ],
                            av[h][0:64, :],
                            rbs[h][:],
                        )
                # final j: fused st tile, quarter DMAs fire per copy
                stf = osb.tile([P, NDC, NQ], BF16, tag="stf", name="stf", bufs=1)
                qo = (NJ - 1) * NQ
                for np2 in range(4):
                    ops = scp.tile([P, 2, NQ], F32, tag="sc", name="op")
                    for i in range(2):
                        n = np2 * 2 + i
                        for c in range(EC):
                            nc.tensor.matmul(
                                ops[:, i, :],
                                r(wo_sb[:, c, n * P : (n + 1) * P]),
                                r(aT[c][:, qo : qo + NQ]),
                                start=(c == 0),
                                stop=(c == EC - 1),
                            )
                    nc.scalar.copy(stf[:, np2 * 2 : np2 * 2 + 1, :], ops[:, 0:1, :])
                    nc.vector.tensor_copy(stf[:, np2 * 2 + 1 : np2 * 2 + 2, :], ops[:, 1:2, :])
                    nc.sync.dma_start(
                        out[np2 * 2 * P : (np2 + 1) * 2 * P, qo : qo + NQ].rearrange(
                            "(i p) q -> p i q", i=2
                        ),
                        stf[:, np2 * 2 : np2 * 2 + 2, :],
                    )

    nc.compile()
    return nc


def make_in_maps(queries, keys, values, Wq, bq, Wk, bk, Wv, bv, Wo, bo):
    """Build per-core input maps. core = b*4 + g."""
    f32 = np.float32
    import ml_dtypes
    x_dt = ml_dtypes.bfloat16
    in_maps = []
    for core in range(8):
        b, g = core // 4, core % 4
        cols = slice(g * EPC, (g + 1) * EPC)
        off = 2 ** g
        ramp = (np.arange(P) - off).astype(f32)
        ramp_last = ramp.copy()
        ramp_last[P - 1] = 1e9  # s == L-1 always visible
        in_maps.append(
            {
                "xqT": np.ascontiguousarray(queries[b].T).astype(x_dt),
                "xkT": np.ascontiguousarray(keys[b].T).astype(x_dt),
                "xvT": np.ascontiguousarray(values[b].T).astype(x_dt),
                "wqT": np.ascontiguousarray(Wq[cols, :].T).astype(x_dt),
                "wkT": np.ascontiguousarray(Wk[cols, :].T).astype(x_dt),
                "wvT": np.ascontiguousarray(Wv[cols, :].T).astype(x_dt),
                "woT": np.ascontiguousarray(Wo[:, cols].T, dtype=f32),
                "bqk": np.stack(
                    [bq[cols].reshape(EC, P), bk[cols].reshape(EC, P)]
                ).astype(f32),
                "ramps": np.stack([ramp, ramp_last]),
            }
        )
    return in_maps


def gather_outputs(results, Wo, bv, bo):
    """results: list of 8 dicts with 'outT' [D, L] bf16. Returns [2, L, D] f32."""
    host_bias = (Wo.astype(np.float64) @ bv.astype(np.float64) + bo).astype(
        np.float32
    )
    out = np.zeros((2, L, D), np.float32)
    for b in range(2):
        acc = np.zeros((D, L), np.float32)
        for g in range(4):
            acc += np.asarray(results[b * 4 + g]["outT"]).astype(np.float32)
        out[b] = acc.T + host_bias[None, :]
    return out




# ======================= host entry point =======================
_NC_CACHE = None


def kernel(queries, keys, values, Wq, bq, Wk, bk, Wv, bv, Wo, bo):
    """Full-input entry: shards across 8 NeuronCores, returns [2, 2048, 1024]."""
    global _NC_CACHE
    from concourse.bass_utils import run_bass_kernel_spmd

    args = [np.asarray(a) for a in (queries, keys, values, Wq, bq, Wk, bk, Wv, bv, Wo, bo)]
    queries, keys, values, Wq, bq, Wk, bk, Wv, bv, Wo, bo = args
    if _NC_CACHE is None:
        _NC_CACHE = build_nc()
    in_maps = make_in_maps(queries, keys, values, Wq, bq, Wk, bk, Wv, bv, Wo, bo)
    res = run_bass_kernel_spmd(_NC_CACHE, in_maps, list(range(8)))
    return gather_outputs(res.results, Wo, bv, bo)


# revision 40
# speedup vs baseline: 1.0348x; 1.0097x over previous
"""Sparse diag-masked multi-head attention layer on 8 trn2 cores.

Sharding: core = b*4 + g  (b in 0..1 batches, g in 0..3 head-groups).
Each core computes heads 4g..4g+3 of batch b. Head-group g has band
offset off = 2**g: visible(q, s) <=> s >= q + off  OR  s == L-1.

v2 layout (all matmul operands bf16, PSUM f32):
  qT, kT  [e=256, L] bf16   (2 sbuf tiles of [128, L])   e on partitions
  v_pack  16 x [128 s, 4 h, 66] bf16  (cols 0:64 v, col 64 ones)
  scoresT [s 128, q ev] psum = kT_chunk.T @ qT_chunk  (contract e=64)
  P = exp(0.125*scoresT) -> sbuf bf16, band-masked via (iota<=T[p])*P
      (iota + thresholds in fp16 so the DVE mask runs in 4x mode)
  num/den: psum_o[h] [65, ev] += v_aug.T @ P   (contract s)
  attnT   [e, q] bf16 = num * bcast(1/den)  (DVE reads psum directly)
  outT    [n, q] psum = woT_chunk.T @ attnT_chunk; DVE copy -> bf16 st
  out-projection of chunk j is interleaved into chunk j+1's score loop
  to keep the PE fed while the normalize chain drains.

Host: out[b] = sum_g outT(b,g).T + bv @ Wo.T + bo  (f32 accumulate)
"""
import sys

sys.path.insert(0, "/opt/trn_rl_repo")

import numpy as np

import concourse.bacc as bacc
import concourse.bass as bass
import concourse.mybir as mybir
import concourse.tile as tile

P = 128
L = 2048
D = 1024
EPC = 256  # head-dims per core (4 heads x 64)
EC = 2  # e-chunks of 128
HPC = 4  # heads per core
NQ = 512  # q-chunk width
NJ = L // NQ  # 4
NSB = L // P  # 16 s-blocks
NDC = D // P  # 8 d-chunks
SCALE = 0.125  # 1/sqrt(64)

F32 = mybir.dt.float32
F32R = mybir.dt.float32r
BF16 = mybir.dt.bfloat16
F16 = mybir.dt.float16
X_DT = BF16  # dtype of streamed activations + qkv weights (DMA halving)


def r(ap):
    return ap.bitcast(F32R)


def build_nc():
    nc = bacc.Bacc("TRN2", target_bir_lowering=False, debug=False)

    xq = nc.dram_tensor("xqT", [D, L], X_DT, kind="ExternalInput")
    xk = nc.dram_tensor("xkT", [D, L], X_DT, kind="ExternalInput")
    xv = nc.dram_tensor("xvT", [D, L], X_DT, kind="ExternalInput")
    wq = nc.dram_tensor("wqT", [D, EPC], X_DT, kind="ExternalInput")
    wk = nc.dram_tensor("wkT", [D, EPC], X_DT, kind="ExternalInput")
    wv = nc.dram_tensor("wvT", [D, EPC], X_DT, kind="ExternalInput")
    wo = nc.dram_tensor("woT", [EPC, D], F32, kind="ExternalInput")
    bqk = nc.dram_tensor("bqk", [2, EC, P], F32, kind="ExternalInput")
    ramps = nc.dram_tensor("ramps", [2, P], F32, kind="ExternalInput")
    out = nc.dram_tensor("outT", [D, L], BF16, kind="ExternalOutput")

    with tile.TileContext(nc) as tc:
        with (
            tc.tile_pool(name="consts", bufs=1) as consts,
            tc.tile_pool(name="acts", bufs=1) as acts,
        ):
            # ---- constants / weights
            wq_sb = consts.tile([P, NDC, EPC], X_DT, tag="wq")
            wk_sb = consts.tile([P, NDC, EPC], X_DT, tag="wk")
            wv_sb = consts.tile([P, NDC, EPC], X_DT, tag="wv")
            wv_r = wv.rearrange("(dc p) e -> p dc e", p=P)
            nc.sync.dma_start(wv_sb[:, 0:4, :], wv_r[:, 0:4, :])
            wo_sb = consts.tile([P, EC, D], F32R, tag="wo")
            wo_raw = consts.tile([P, EC, D], F32, tag="wo_raw")
            bqk_sb = consts.tile([P, 2, EC], F32, tag="bqk")
            ramps_sb = consts.tile([P, 2], F32, tag="ramps")

            ones_col = consts.tile([P, 1], F32, tag="ones_col")
            nc.vector.memset(ones_col[:], 1.0)
            c0i = consts.tile([P, NQ], mybir.dt.int32, tag="c0i")
            nc.gpsimd.iota(c0i[:], pattern=[[1, NQ]], base=0, channel_multiplier=0)
            c0h = consts.tile([P, NQ], F32, tag="c0h")
            nc.vector.tensor_copy(c0h[:], c0i[:])

            # ---- persistent activations
            qT = [acts.tile([P, L], F32R, tag=f"qT{c}", name=f"qT{c}") for c in range(EC)]
            kT = [acts.tile([P, L], F32R, tag=f"kT{c}", name=f"kT{c}") for c in range(EC)]
            aT = [acts.tile([P, L], F32R, tag=f"aT{c}", name=f"aT{c}") for c in range(EC)]
            vpk = [acts.tile([P, HPC, 66], F32R, tag=f"v{sb}", name=f"v{sb}") for sb in range(NSB)]

            # ================= phase 1: projections =================
            with (
                tc.tile_pool(name="xs", bufs=4) as xpool,
                tc.tile_pool(name="pjp", bufs=4, space=bass.MemorySpace.PSUM) as pjp,
            ):
                # v projection first (attention needs all of vpk; also the
                # heaviest DMA). Quad-grouped x tiles: one DMA feeds 4 s-blocks.
                xv_r = xv.rearrange("(dc p) l -> p dc l", p=P)
                for qd in range(NSB // 4 - 1, -1, -1):
                    xt = xpool.tile([P, NDC, 4 * P], X_DT, tag="xv", name="xv", bufs=3)
                    if qd == NSB // 4 - 1:
                        for dc in range(NDC):
                            nc.sync.dma_start(
                                xt[:, dc, :],
                                xv_r[:, dc, qd * 4 * P : (qd + 1) * 4 * P],
                            )
                            if dc == 3:
                                nc.sync.dma_start(wv_sb[:, 4:8, :], wv_r[:, 4:8, :])
                    else:
                        nc.sync.dma_start(
                            xt[:], xv_r[:, :, qd * 4 * P : (qd + 1) * 4 * P]
                        )
                    for si in range(3, -1, -1):
                        sb = qd * 4 + si
                        psv = pjp.tile([P, EPC], F32, tag="vps", name="vps", bufs=2)
                        for dc in range(NDC):
                            nc.tensor.matmul(
                                psv[:],
                                xt[:, dc, si * P : (si + 1) * P],
                                wv_sb[:, dc, :],
                                start=(dc == 0),
                                stop=(dc == NDC - 1),
                            )
                        nc.scalar.copy(
                            vpk[sb][:, :, 0:64],
                            psv[:].rearrange("p (h e) -> p h e", e=64),
                        )
                        nc.vector.tensor_copy(
                            vpk[sb][:, :, 64:65],
                            ones_col[:, None, :].broadcast_to([P, HPC, 1]),
                        )

                nc.sync.dma_start(bqk_sb[:], bqk.rearrange("t c p -> p t c"))
                nc.sync.dma_start(ramps_sb[:], ramps.rearrange("t p -> p t"))

                for ti, (xdram, wsb) in enumerate(((xk, wk_sb), (xq, wq_sb))):
                    dst = (kT, qT)[ti]
                    bias_t = 1 - ti  # bqk row 0 = bq, row 1 = bk
                    wdram = (wk, wq)[ti]
                    nc.sync.dma_start(wsb[:], wdram.rearrange("(dc p) e -> p dc e", p=P))
                    for lp in ((1, 0) if ti == 0 else (0, 1)):  # k: tail first
                        ps = [
                            [pjp.tile([P, NQ], F32, tag="pj", name="pj", bufs=6) for _ in range(2)]
                            for _ in range(EC)
                        ]
                        for dc in range(NDC):
                            xt = xpool.tile([P, 2 * NQ], X_DT, tag=f"x{ti}", name="xt", bufs=8)
                            nc.sync.dma_start(
                                xt[:],
                                xdram[
                                    dc * P : (dc + 1) * P,
                                    lp * 2 * NQ : (lp + 1) * 2 * NQ,
                                ],
                            )
                            for ec in range(EC):
                                for l2 in range(2):
                                    nc.tensor.matmul(
                                        ps[ec][l2][:],
                                        wsb[:, dc, ec * P : (ec + 1) * P],
                                        xt[:, l2 * NQ : (l2 + 1) * NQ],
                                        start=(dc == 0),
                                        stop=(dc == NDC - 1),
                                    )
                        for l2 in range(2):
                            for ec in range(EC):
                                lc = lp * 2 + l2
                                # split each psum pair across Act/DVE so the
                                # last adds (gating the pool transition) drain
                                # in parallel; q's first chunk fully on Act
                                on_act = ec == 0 or (ti == 1 and lp == 0 and l2 == 0)
                                if on_act:
                                    nc.scalar.activation(
                                        dst[ec][:, lc * NQ : (lc + 1) * NQ],
                                        ps[ec][l2][:],
                                        mybir.ActivationFunctionType.Identity,
                                        bias=bqk_sb[:, bias_t, ec : ec + 1],
                                        scale=1.0,
                                    )
                                else:
                                    nc.vector.tensor_scalar_add(
                                        dst[ec][:, lc * NQ : (lc + 1) * NQ],
                                        ps[ec][l2][:],
                                        bqk_sb[:, bias_t, ec : ec + 1],
                                    )

                nc.sync.dma_start(wo_raw[:], wo.rearrange("(c p) n -> p c n", p=P))
                nc.gpsimd.tensor_copy(wo_sb[:], wo_raw[:])

            # ================= phase 2: attention =================
            with (
                tc.tile_pool(name="scp", bufs=2, space=bass.MemorySpace.PSUM) as scp,
                tc.tile_pool(name="avp", bufs=4, space=bass.MemorySpace.PSUM) as avp,
                tc.tile_pool(name="pp", bufs=7) as pp,
                tc.tile_pool(name="tp", bufs=4) as tp,
                tc.tile_pool(name="rdp", bufs=4) as rdp,
                tc.tile_pool(name="rbp", bufs=4) as rbp,
                tc.tile_pool(name="osb", bufs=4) as osb,
            ):
                def emit_op_group(jo, np2, st_on_act=False):
                    # out-projection columns [np2*256, np2*256+256) for q-chunk jo
                    qo = jo * NQ
                    ops = scp.tile([P, 2, NQ], F32, tag="sc", name="op")
                    for i in range(2):
                        n = np2 * 2 + i
                        for c in range(EC):
                            nc.tensor.matmul(
                                ops[:, i, :],
                                r(wo_sb[:, c, n * P : (n + 1) * P]),
                                r(aT[c][:, qo : qo + NQ]),
                                start=(c == 0),
                                stop=(c == EC - 1),
                            )
                    st = osb.tile([P, 2, NQ], BF16, tag="ost", name="ost")
                    if st_on_act:
                        nc.scalar.copy(st[:], ops[:])
                    else:
                        nc.vector.tensor_copy(st[:], ops[:])
                    nc.sync.dma_start(
                        out[np2 * 2 * P : (np2 + 1) * 2 * P, qo : qo + NQ].rearrange(
                            "(i p) q -> p i q", i=2
                        ),
                        st[:],
                    )

                for j in range(NJ):
                    q0 = j * NQ
                    js_lo = 4 * j
                    av = [avp.tile([65, NQ], F32, tag="av", name="av") for _ in range(HPC)]
                    # js descending: accumulation group starts at js=15 with a
                    # full-width MM; diagonal tiles then accumulate a prefix.
                    for js in range(NSB - 1, js_lo - 1, -1):
                        s0 = js * P
                        k = js - js_lo
                        masked = k <= 4
                        # visible q-prefix (min 256: f32r matmuls below N=256 run at 1/4 rate)
                        ev = min(NQ, max(2 * P, (k + 1) * P))
                        # exp only the truly visible prefix; for k=0 the mask
                        # zeroes cols 128:256 (stale-but-finite pool content)
                        ew = min(ev, (k + 1) * P)
                        tt = None
                        for pr in range(2):  # head pairs (0,1), (2,3)
                            sp = scp.tile([P, 2, NQ], F32, tag="sc", name="sc")
                            for i in range(2):
                                nc.tensor.matmul(
                                    sp[:, i, 0:ev],
                                    r(kT[pr][i * 64 : i * 64 + 64, s0 : s0 + P]),
                                    r(qT[pr][i * 64 : i * 64 + 64, q0 : q0 + ev]),
                                    start=True,
                                    stop=True,
                                )
                            pt = pp.tile([P, 2, NQ], F32R, tag="p", name="p")
                            nc.scalar.activation(
                                pt[:, :, 0:ew],
                                sp[:, :, 0:ew],
                                mybir.ActivationFunctionType.Exp,
                                scale=SCALE,
                            )
                            if masked:
                                if tt is None:
                                    tt = tp.tile([P, 1], F32, tag="t", name="t")
                                    ramp = ramps_sb[:, 1:2] if js == NSB - 1 else ramps_sb[:, 0:1]
                                    nc.vector.tensor_scalar_add(
                                        tt[:], ramp, float(s0 - q0)
                                    )
                                m0 = max(0, k * P - 8)
                                nc.vector.scalar_tensor_tensor(
                                    pt[:, :, m0:ev],
                                    c0h[:, None, m0:ev].broadcast_to([P, 2, ev - m0]),
                                    tt[:],
                                    pt[:, :, m0:ev],
                                    op0=mybir.AluOpType.is_le,
                                    op1=mybir.AluOpType.mult,
                                )
                            for i in range(2):
                                h = 2 * pr + i
                                nc.tensor.matmul(
                                    av[h][:, 0:ev],
                                    r(vpk[js][:, h, 0:65]),
                                    r(pt[:, i, 0:ev]),
                                    start=(js == NSB - 1),
                                    stop=(js == js_lo),
                                )
                    # out projection of the previous j-chunk, emitted
                    # BEFORE this j's normalize: its DVE st copies drain ahead
                    # of the normalize chain so score-psum slots free early.
                    # The last j normalizes first (no next-j work hides it) and
                    # its predecessor's sts go to the idle Act engine instead.
                    if 0 < j < NJ - 1:
                        for np2 in range(4 if j == 1 else 2):
                            emit_op_group(j - 1, np2, st_on_act=True)
                    if j == NJ - 1:
                        for np2 in range(4):
                            emit_op_group(j - 1, np2, st_on_act=True)
                        # deferred ops(1) groups: PE filler while normalize(3)
                        # drains on DVE/Pool (nothing else remains at the tail)
                        for np2 in (2, 3):
                            emit_op_group(1, np2, st_on_act=True)
                    # all recips first: DVE runs them back-to-back while
                    # the Pool broadcasts pipeline behind them; muls last
                    rds, rbs = [], []
                    for h in range(HPC):
                        rd = rdp.tile([1, NQ], F32, tag="rd", bufs=4)
                        nc.vector.reciprocal(rd[:], av[h][64:65, :])
                        rds.append(rd)
                    for h in range(HPC):
                        rb = rbp.tile([64, NQ], F32, tag="rb", bufs=4)
                        nc.gpsimd.partition_broadcast(rb[:], rds[h][:], channels=64)
                        rbs.append(rb)
                    for h in range(HPC):
                        ec, r0 = h // 2, (h % 2) * 64
                        nc.vector.tensor_mul(
                            aT[ec][r0 : r0 + 64, q0 : q0 + NQ],
                            av[h][0:64, :],
                            rbs[h][:],
                        )
                # final j: fused st tile, quarter DMAs fire per copy
                stf = osb.tile([P, NDC, NQ], BF16, tag="stf", name="stf", bufs=1)
                qo = (NJ - 1) * NQ
                for np2 in range(4):
                    ops = scp.tile([P, 2, NQ], F32, tag="sc", name="op")
                    for i in range(2):
                        n = np2 * 2 + i
                        for c in range(EC):
                            nc.tensor.matmul(
                                ops[:, i, :],
                                r(wo_sb[:, c, n * P : (n + 1) * P]),
                                r(aT[c][:, qo : qo + NQ]),
                                start=(c == 0),
                                stop=(c == EC - 1),
                            )
                    nc.scalar.copy(stf[:, np2 * 2 : np2 * 2 + 1, :], ops[:, 0:1, :])
                    nc.vector.tensor_copy(stf[:, np2 * 2 + 1 : np2 * 2 + 2, :], ops[:, 1:2, :])
                    nc.sync.dma_start(
                        out[np2 * 2 * P : (np2 + 1) * 2 * P, qo : qo + NQ].rearrange(
                            "(i p) q -> p i q", i=2
                        ),
                        stf[:, np2 * 2 : np2 * 2 + 2, :],
                    )

    nc.compile()
    return nc


def make_in_maps(queries, keys, values, Wq, bq, Wk, bk, Wv, bv, Wo, bo):
    """Build per-core input maps. core = b*4 + g."""
    f32 = np.float32
    import ml_dtypes
    x_dt = ml_dtypes.bfloat16
    in_maps = []
    for core in range(8):
        b, g = core // 4, core % 4
        cols = slice(g * EPC, (g + 1) * EPC)
        off = 2 ** g
        ramp = (np.arange(P) - off).astype(f32)
        ramp_last = ramp.copy()
        ramp_last[P - 1] = 1e9  # s == L-1 always visible
        in_maps.append(
            {
                "xqT": np.ascontiguousarray(queries[b].T).astype(x_dt),
                "xkT": np.ascontiguousarray(keys[b].T).astype(x_dt),
                "xvT": np.ascontiguousarray(values[b].T).astype(x_dt),
                "wqT": np.ascontiguousarray(Wq[cols, :].T).astype(x_dt),
                "wkT": np.ascontiguousarray(Wk[cols, :].T).astype(x_dt),
                "wvT": np.ascontiguousarray(Wv[cols, :].T).astype(x_dt),
                "woT": np.ascontiguousarray(Wo[:, cols].T, dtype=f32),
                "bqk": np.stack(
                    [bq[cols].reshape(EC, P), bk[cols].reshape(EC, P)]
                ).astype(f32),
                "ramps": np.stack([ramp, ramp_last]),
            }
        )
    return in_maps


def gather_outputs(results, Wo, bv, bo):
    """results: list of 8 dicts with 'outT' [D, L] bf16. Returns [2, L, D] f32."""
    host_bias = (Wo.astype(np.float64) @ bv.astype(np.float64) + bo).astype(
        np.float32
    )
    out = np.zeros((2, L, D), np.float32)
    for b in range(2):
        acc = np.zeros((D, L), np.float32)
        for g in range(4):
            acc += np.asarray(results[b * 4 + g]["outT"]).astype(np.float32)
        out[b] = acc.T + host_bias[None, :]
    return out




# ======================= host entry point =======================
_NC_CACHE = None


def kernel(queries, keys, values, Wq, bq, Wk, bk, Wv, bv, Wo, bo):
    """Full-input entry: shards across 8 NeuronCores, returns [2, 2048, 1024]."""
    global _NC_CACHE
    from concourse.bass_utils import run_bass_kernel_spmd

    args = [np.asarray(a) for a in (queries, keys, values, Wq, bq, Wk, bk, Wv, bv, Wo, bo)]
    queries, keys, values, Wq, bq, Wk, bk, Wv, bv, Wo, bo = args
    if _NC_CACHE is None:
        _NC_CACHE = build_nc()
    in_maps = make_in_maps(queries, keys, values, Wq, bq, Wk, bk, Wv, bv, Wo, bo)
    res = run_bass_kernel_spmd(_NC_CACHE, in_maps, list(range(8)))
    return gather_outputs(res.results, Wo, bv, bo)


# revision 48
# speedup vs baseline: 1.0422x; 1.0072x over previous
"""Sparse diag-masked multi-head attention layer on 8 trn2 cores.

Sharding: core = b*4 + g  (b in 0..1 batches, g in 0..3 head-groups).
Each core computes heads 4g..4g+3 of batch b. Head-group g has band
offset off = 2**g: visible(q, s) <=> s >= q + off  OR  s == L-1.

v2 layout (all matmul operands bf16, PSUM f32):
  qT, kT  [e=256, L] bf16   (2 sbuf tiles of [128, L])   e on partitions
  v_pack  16 x [128 s, 4 h, 66] bf16  (cols 0:64 v, col 64 ones)
  scoresT [s 128, q ev] psum = kT_chunk.T @ qT_chunk  (contract e=64)
  P = exp(0.125*scoresT) -> sbuf bf16, band-masked via (iota<=T[p])*P
      (iota + thresholds in fp16 so the DVE mask runs in 4x mode)
  num/den: psum_o[h] [65, ev] += v_aug.T @ P   (contract s)
  attnT   [e, q] bf16 = num * bcast(1/den)  (DVE reads psum directly)
  outT    [n, q] psum = woT_chunk.T @ attnT_chunk; DVE copy -> bf16 st
  out-projection of chunk j is interleaved into chunk j+1's score loop
  to keep the PE fed while the normalize chain drains.

Host: out[b] = sum_g outT(b,g).T + bv @ Wo.T + bo  (f32 accumulate)
"""
import sys

sys.path.insert(0, "/opt/trn_rl_repo")

import numpy as np

import concourse.bacc as bacc
import concourse.bass as bass
import concourse.mybir as mybir
import concourse.tile as tile

P = 128
L = 2048
D = 1024
EPC = 256  # head-dims per core (4 heads x 64)
EC = 2  # e-chunks of 128
HPC = 4  # heads per core
NQ = 512  # q-chunk width
NJ = L // NQ  # 4
NSB = L // P  # 16 s-blocks
NDC = D // P  # 8 d-chunks
SCALE = 0.125  # 1/sqrt(64)

F32 = mybir.dt.float32
F32R = mybir.dt.float32r
BF16 = mybir.dt.bfloat16
F16 = mybir.dt.float16
X_DT = BF16  # dtype of streamed activations + qkv weights (DMA halving)


def r(ap):
    return ap.bitcast(F32R)


def build_nc():
    nc = bacc.Bacc("TRN2", target_bir_lowering=False, debug=False)

    xq = nc.dram_tensor("xqT", [D, L], X_DT, kind="ExternalInput")
    xk = nc.dram_tensor("xkT", [D, L], X_DT, kind="ExternalInput")
    xv = nc.dram_tensor("xvT", [D, L], X_DT, kind="ExternalInput")
    wq = nc.dram_tensor("wqT", [D, EPC], X_DT, kind="ExternalInput")
    wk = nc.dram_tensor("wkT", [D, EPC], X_DT, kind="ExternalInput")
    wv = nc.dram_tensor("wvT", [D, EPC], X_DT, kind="ExternalInput")
    wo = nc.dram_tensor("woT", [EPC, D], F32, kind="ExternalInput")
    bqk = nc.dram_tensor("bqk", [2, EC, P], F32, kind="ExternalInput")
    ramps = nc.dram_tensor("ramps", [2, P], F32, kind="ExternalInput")
    out = nc.dram_tensor("outT", [D, L], BF16, kind="ExternalOutput")

    with tile.TileContext(nc) as tc:
        with (
            tc.tile_pool(name="consts", bufs=1) as consts,
            tc.tile_pool(name="acts", bufs=1) as acts,
        ):
            # ---- constants / weights
            wq_sb = consts.tile([P, NDC, EPC], X_DT, tag="wq")
            wk_sb = consts.tile([P, NDC, EPC], X_DT, tag="wk")
            wv_sb = consts.tile([P, NDC, EPC], X_DT, tag="wv")
            wv_r = wv.rearrange("(dc p) e -> p dc e", p=P)
            nc.sync.dma_start(wv_sb[:, 0:4, :], wv_r[:, 0:4, :])
            wo_sb = consts.tile([P, EC, D], F32R, tag="wo")
            wo_raw = consts.tile([P, EC, D], F32, tag="wo_raw")
            bqk_sb = consts.tile([P, 2, EC], F32, tag="bqk")
            ramps_sb = consts.tile([P, 2], F32, tag="ramps")

            ones_col = consts.tile([P, 1], F32, tag="ones_col")
            nc.vector.memset(ones_col[:], 1.0)
            c0i = consts.tile([P, NQ], mybir.dt.int32, tag="c0i")
            nc.gpsimd.iota(c0i[:], pattern=[[1, NQ]], base=0, channel_multiplier=0)
            c0h = consts.tile([P, NQ], F32, tag="c0h")
            nc.vector.tensor_copy(c0h[:], c0i[:])

            # ---- persistent activations
            qT = [acts.tile([P, L], F32R, tag=f"qT{c}", name=f"qT{c}") for c in range(EC)]
            kT = [acts.tile([P, L], F32R, tag=f"kT{c}", name=f"kT{c}") for c in range(EC)]
            aT = [acts.tile([P, L], F32R, tag=f"aT{c}", name=f"aT{c}") for c in range(EC)]
            vpk = [acts.tile([P, HPC, 66], F32R, tag=f"v{sb}", name=f"v{sb}") for sb in range(NSB)]

            # ================= phase 1: projections =================
            with (
                tc.tile_pool(name="xs", bufs=4) as xpool,
                tc.tile_pool(name="pjp", bufs=4, space=bass.MemorySpace.PSUM) as pjp,
            ):
                # v projection first (attention needs all of vpk; also the
                # heaviest DMA). Quad-grouped x tiles: one DMA feeds 4 s-blocks.
                xv_r = xv.rearrange("(dc p) l -> p dc l", p=P)
                for qd in range(NSB // 4 - 1, -1, -1):
                    xt = xpool.tile([P, NDC, 4 * P], X_DT, tag="xv", name="xv", bufs=3)
                    if qd == NSB // 4 - 1:
                        for dc in range(NDC):
                            nc.sync.dma_start(
                                xt[:, dc, :],
                                xv_r[:, dc, qd * 4 * P : (qd + 1) * 4 * P],
                            )
                            if dc == 3:
                                nc.sync.dma_start(wv_sb[:, 4:8, :], wv_r[:, 4:8, :])
                    elif qd == NSB // 4 - 2:
                        # halves in compute order (si descending) so the PE
                        # can start on the second pair while the first streams
                        nc.sync.dma_start(
                            xt[:, :, 2 * P : 4 * P],
                            xv_r[:, :, qd * 4 * P + 2 * P : (qd + 1) * 4 * P],
                        )
                        nc.sync.dma_start(
                            xt[:, :, 0 : 2 * P],
                            xv_r[:, :, qd * 4 * P : qd * 4 * P + 2 * P],
                        )
                    else:
                        nc.sync.dma_start(
                            xt[:], xv_r[:, :, qd * 4 * P : (qd + 1) * 4 * P]
                        )
                    for si in range(3, -1, -1):
                        sb = qd * 4 + si
                        psv = pjp.tile([P, EPC], F32, tag="vps", name="vps", bufs=2)
                        for dc in range(NDC):
                            nc.tensor.matmul(
                                psv[:],
                                xt[:, dc, si * P : (si + 1) * P],
                                wv_sb[:, dc, :],
                                start=(dc == 0),
                                stop=(dc == NDC - 1),
                            )
                        nc.scalar.copy(
                            vpk[sb][:, :, 0:64],
                            psv[:].rearrange("p (h e) -> p h e", e=64),
                        )
                        nc.vector.tensor_copy(
                            vpk[sb][:, :, 64:65],
                            ones_col[:, None, :].broadcast_to([P, HPC, 1]),
                        )

                nc.sync.dma_start(bqk_sb[:], bqk.rearrange("t c p -> p t c"))
                nc.sync.dma_start(ramps_sb[:], ramps.rearrange("t p -> p t"))

                for ti, (xdram, wsb) in enumerate(((xk, wk_sb), (xq, wq_sb))):
                    dst = (kT, qT)[ti]
                    bias_t = 1 - ti  # bqk row 0 = bq, row 1 = bk
                    wdram = (wk, wq)[ti]
                    nc.sync.dma_start(wsb[:], wdram.rearrange("(dc p) e -> p dc e", p=P))
                    for lp in ((1, 0) if ti == 0 else (0, 1)):  # k: tail first
                        ps = [
                            [pjp.tile([P, NQ], F32, tag="pj", name="pj", bufs=6) for _ in range(2)]
                            for _ in range(EC)
                        ]
                        for dc in range(NDC):
                            xt = xpool.tile([P, 2 * NQ], X_DT, tag=f"x{ti}", name="xt", bufs=8)
                            nc.sync.dma_start(
                                xt[:],
                                xdram[
                                    dc * P : (dc + 1) * P,
                                    lp * 2 * NQ : (lp + 1) * 2 * NQ,
                                ],
                            )
                            for ec in range(EC):
                                for l2 in range(2):
                                    nc.tensor.matmul(
                                        ps[ec][l2][:],
                                        wsb[:, dc, ec * P : (ec + 1) * P],
                                        xt[:, l2 * NQ : (l2 + 1) * NQ],
                                        start=(dc == 0),
                                        stop=(dc == NDC - 1),
                                    )
                        for l2 in range(2):
                            for ec in range(EC):
                                lc = lp * 2 + l2
                                # split each psum pair across Act/DVE so the
                                # last adds (gating the pool transition) drain
                                # in parallel; q's first chunk fully on Act
                                on_act = ec == 0 or (ti == 1 and lp == 0 and l2 == 0)
                                if on_act:
                                    nc.scalar.activation(
                                        dst[ec][:, lc * NQ : (lc + 1) * NQ],
                                        ps[ec][l2][:],
                                        mybir.ActivationFunctionType.Identity,
                                        bias=bqk_sb[:, bias_t, ec : ec + 1],
                                        scale=1.0,
                                    )
                                else:
                                    nc.vector.tensor_scalar_add(
                                        dst[ec][:, lc * NQ : (lc + 1) * NQ],
                                        ps[ec][l2][:],
                                        bqk_sb[:, bias_t, ec : ec + 1],
                                    )

                nc.sync.dma_start(wo_raw[:], wo.rearrange("(c p) n -> p c n", p=P))
                nc.gpsimd.tensor_copy(wo_sb[:], wo_raw[:])

            # ================= phase 2: attention =================
            with (
                tc.tile_pool(name="scp", bufs=2, space=bass.MemorySpace.PSUM) as scp,
                tc.tile_pool(name="avp", bufs=4, space=bass.MemorySpace.PSUM) as avp,
                tc.tile_pool(name="pp", bufs=7) as pp,
                tc.tile_pool(name="tp", bufs=4) as tp,
                tc.tile_pool(name="rdp", bufs=4) as rdp,
                tc.tile_pool(name="rbp", bufs=4) as rbp,
                tc.tile_pool(name="osb", bufs=4) as osb,
            ):
                def emit_op_group(jo, np2, st_on_act=False):
                    # out-projection columns [np2*256, np2*256+256) for q-chunk jo
                    qo = jo * NQ
                    ops = scp.tile([P, 2, NQ], F32, tag="sc", name="op")
                    for i in range(2):
                        n = np2 * 2 + i
                        for c in range(EC):
                            nc.tensor.matmul(
                                ops[:, i, :],
                                r(wo_sb[:, c, n * P : (n + 1) * P]),
                                r(aT[c][:, qo : qo + NQ]),
                                start=(c == 0),
                                stop=(c == EC - 1),
                            )
                    st = osb.tile([P, 2, NQ], BF16, tag="ost", name="ost")
                    if st_on_act:
                        nc.scalar.copy(st[:], ops[:])
                    else:
                        nc.vector.tensor_copy(st[:], ops[:])
                    nc.sync.dma_start(
                        out[np2 * 2 * P : (np2 + 1) * 2 * P, qo : qo + NQ].rearrange(
                            "(i p) q -> p i q", i=2
                        ),
                        st[:],
                    )

                for j in range(NJ):
                    q0 = j * NQ
                    js_lo = 4 * j
                    av = [avp.tile([65, NQ], F32, tag="av", name="av") for _ in range(HPC)]
                    # js descending: accumulation group starts at js=15 with a
                    # full-width MM; diagonal tiles then accumulate a prefix.
                    for js in range(NSB - 1, js_lo - 1, -1):
                        s0 = js * P
                        k = js - js_lo
                        masked = k <= 4
                        # visible q-prefix (min 256: f32r matmuls below N=256 run at 1/4 rate)
                        ev = min(NQ, max(2 * P, (k + 1) * P))
                        # exp only the truly visible prefix; for k=0 the mask
                        # zeroes cols 128:256 (stale-but-finite pool content)
                        ew = min(ev, (k + 1) * P)
                        tt = None
                        for pr in range(2):  # head pairs (0,1), (2,3)
                            sp = scp.tile([P, 2, NQ], F32, tag="sc", name="sc")
                            for i in range(2):
                                nc.tensor.matmul(
                                    sp[:, i, 0:ev],
                                    r(kT[pr][i * 64 : i * 64 + 64, s0 : s0 + P]),
                                    r(qT[pr][i * 64 : i * 64 + 64, q0 : q0 + ev]),
                                    start=True,
                                    stop=True,
                                )
                            pt = pp.tile([P, 2, NQ], F32R, tag="p", name="p")
                            nc.scalar.activation(
                                pt[:, :, 0:ew],
                                sp[:, :, 0:ew],
                                mybir.ActivationFunctionType.Exp,
                                scale=SCALE,
                            )
                            if masked:
                                if tt is None:
                                    tt = tp.tile([P, 1], F32, tag="t", name="t")
                                    ramp = ramps_sb[:, 1:2] if js == NSB - 1 else ramps_sb[:, 0:1]
                                    nc.vector.tensor_scalar_add(
                                        tt[:], ramp, float(s0 - q0)
                                    )
                                m0 = max(0, k * P - 8)
                                nc.vector.scalar_tensor_tensor(
                                    pt[:, :, m0:ev],
                                    c0h[:, None, m0:ev].broadcast_to([P, 2, ev - m0]),
                                    tt[:],
                                    pt[:, :, m0:ev],
                                    op0=mybir.AluOpType.is_le,
                                    op1=mybir.AluOpType.mult,
                                )
                            for i in range(2):
                                h = 2 * pr + i
                                nc.tensor.matmul(
                                    av[h][:, 0:ev],
                                    r(vpk[js][:, h, 0:65]),
                                    r(pt[:, i, 0:ev]),
                                    start=(js == NSB - 1),
                                    stop=(js == js_lo),
                                )
                    # out projection of the previous j-chunk, emitted
                    # BEFORE this j's normalize: its DVE st copies drain ahead
                    # of the normalize chain so score-psum slots free early.
                    # The last j normalizes first (no next-j work hides it) and
                    # its predecessor's sts go to the idle Act engine instead.
                    if 0 < j < NJ - 1:
                        for np2 in range(4 if j == 1 else 3):
                            emit_op_group(j - 1, np2, st_on_act=True)
                    if j == NJ - 1:
                        for np2 in range(4):
                            emit_op_group(j - 1, np2, st_on_act=True)
                        # deferred ops(1) group: PE filler while normalize(3)
                        # drains on DVE/Pool (nothing else remains at the tail)
                        for np2 in (3,):
                            emit_op_group(1, np2, st_on_act=True)

                    # all recips first: DVE runs them back-to-back while
                    # the Pool broadcasts pipeline behind them; muls last
                    rds, rbs = [], []
                    for h in range(HPC):
                        rd = rdp.tile([1, NQ], F32, tag="rd", bufs=4)
                        nc.vector.reciprocal(rd[:], av[h][64:65, :])
                        rds.append(rd)
                    for h in range(HPC):
                        rb = rbp.tile([64, NQ], F32, tag="rb", bufs=4)
                        nc.gpsimd.partition_broadcast(rb[:], rds[h][:], channels=64)
                        rbs.append(rb)
                    for h in range(HPC):
                        ec, r0 = h // 2, (h % 2) * 64
                        nc.vector.tensor_mul(
                            aT[ec][r0 : r0 + 64, q0 : q0 + NQ],
                            av[h][0:64, :],
                            rbs[h][:],
                        )
                # final j: fused st tile, quarter DMAs fire per copy
                stf = osb.tile([P, NDC, NQ], BF16, tag="stf", name="stf", bufs=1)
                qo = (NJ - 1) * NQ
                for np2 in range(4):
                    ops = scp.tile([P, 2, NQ], F32, tag="sc", name="op")
                    for i in range(2):
                        n = np2 * 2 + i
                        for c in range(EC):
                            nc.tensor.matmul(
                                ops[:, i, :],
                                r(wo_sb[:, c, n * P : (n + 1) * P]),
                                r(aT[c][:, qo : qo + NQ]),
                                start=(c == 0),
                                stop=(c == EC - 1),
                            )
                    nc.vector.tensor_copy(stf[:, np2 * 2 : np2 * 2 + 2, :], ops[:])
                    nc.sync.dma_start(
                        out[np2 * 2 * P : (np2 + 1) * 2 * P, qo : qo + NQ].rearrange(
                            "(i p) q -> p i q", i=2
                        ),
                        stf[:, np2 * 2 : np2 * 2 + 2, :],
                    )

    nc.compile()
    return nc


def make_in_maps(queries, keys, values, Wq, bq, Wk, bk, Wv, bv, Wo, bo):
    """Build per-core input maps. core = b*4 + g."""
    f32 = np.float32
    import ml_dtypes
    x_dt = ml_dtypes.bfloat16
    in_maps = []
    for core in range(8):
        b, g = core // 4, core % 4
        cols = slice(g * EPC, (g + 1) * EPC)
        off = 2 ** g
        ramp = (np.arange(P) - off).astype(f32)
        ramp_last = ramp.copy()
        ramp_last[P - 1] = 1e9  # s == L-1 always visible
        in_maps.append(
            {
                "xqT": np.ascontiguousarray(queries[b].T).astype(x_dt),
                "xkT": np.ascontiguousarray(keys[b].T).astype(x_dt),
                "xvT": np.ascontiguousarray(values[b].T).astype(x_dt),
                "wqT": np.ascontiguousarray(Wq[cols, :].T).astype(x_dt),
                "wkT": np.ascontiguousarray(Wk[cols, :].T).astype(x_dt),
                "wvT": np.ascontiguousarray(Wv[cols, :].T).astype(x_dt),
                "woT": np.ascontiguousarray(Wo[:, cols].T, dtype=f32),
                "bqk": np.stack(
                    [bq[cols].reshape(EC, P), bk[cols].reshape(EC, P)]
                ).astype(f32),
                "ramps": np.stack([ramp, ramp_last]),
            }
        )
    return in_maps


def gather_outputs(results, Wo, bv, bo):
    """results: list of 8 dicts with 'outT' [D, L] bf16. Returns [2, L, D] f32."""
    host_bias = (Wo.astype(np.float64) @ bv.astype(np.float64) + bo).astype(
        np.float32
    )
    out = np.zeros((2, L, D), np.float32)
    for b in range(2):
        acc = np.zeros((D, L), np.float32)
        for g in range(4):
            acc += np.asarray(results[b * 4 + g]["outT"]).astype(np.float32)
        out[b] = acc.T + host_bias[None, :]
    return out




# ======================= host entry point =======================
_NC_CACHE = None


def kernel(queries, keys, values, Wq, bq, Wk, bk, Wv, bv, Wo, bo):
    """Full-input entry: shards across 8 NeuronCores, returns [2, 2048, 1024]."""
    global _NC_CACHE
    from concourse.bass_utils import run_bass_kernel_spmd

    args = [np.asarray(a) for a in (queries, keys, values, Wq, bq, Wk, bk, Wv, bv, Wo, bo)]
    queries, keys, values, Wq, bq, Wk, bk, Wv, bv, Wo, bo = args
    if _NC_CACHE is None:
        _NC_CACHE = build_nc()
    in_maps = make_in_maps(queries, keys, values, Wq, bq, Wk, bk, Wv, bv, Wo, bo)
    res = run_bass_kernel_spmd(_NC_CACHE, in_maps, list(range(8)))
    return gather_outputs(res.results, Wo, bv, bo)


# revision 54
# speedup vs baseline: 1.0459x; 1.0035x over previous
"""Sparse diag-masked multi-head attention layer on 8 trn2 cores.

Sharding: core = b*4 + g  (b in 0..1 batches, g in 0..3 head-groups).
Each core computes heads 4g..4g+3 of batch b. Head-group g has band
offset off = 2**g: visible(q, s) <=> s >= q + off  OR  s == L-1.

v2 layout (all matmul operands bf16, PSUM f32):
  qT, kT  [e=256, L] bf16   (2 sbuf tiles of [128, L])   e on partitions
  v_pack  16 x [128 s, 4 h, 66] bf16  (cols 0:64 v, col 64 ones)
  scoresT [s 128, q ev] psum = kT_chunk.T @ qT_chunk  (contract e=64)
  P = exp(0.125*scoresT) -> sbuf bf16, band-masked via (iota<=T[p])*P
      (iota + thresholds in fp16 so the DVE mask runs in 4x mode)
  num/den: psum_o[h] [65, ev] += v_aug.T @ P   (contract s)
  attnT   [e, q] bf16 = num * bcast(1/den)  (DVE reads psum directly)
  outT    [n, q] psum = woT_chunk.T @ attnT_chunk; DVE copy -> bf16 st
  out-projection of chunk j is interleaved into chunk j+1's score loop
  to keep the PE fed while the normalize chain drains.

Host: out[b] = sum_g outT(b,g).T + bv @ Wo.T + bo  (f32 accumulate)
"""
import sys

sys.path.insert(0, "/opt/trn_rl_repo")

import numpy as np

import concourse.bacc as bacc
import concourse.bass as bass
import concourse.mybir as mybir
import concourse.tile as tile

P = 128
L = 2048
D = 1024
EPC = 256  # head-dims per core (4 heads x 64)
EC = 2  # e-chunks of 128
HPC = 4  # heads per core
NQ = 512  # q-chunk width
NJ = L // NQ  # 4
NSB = L // P  # 16 s-blocks
NDC = D // P  # 8 d-chunks
SCALE = 0.125  # 1/sqrt(64)

F32 = mybir.dt.float32
F32R = mybir.dt.float32r
BF16 = mybir.dt.bfloat16
F16 = mybir.dt.float16
X_DT = BF16  # dtype of streamed activations + qkv weights (DMA halving)


def r(ap):
    return ap.bitcast(F32R)


def build_nc():
    nc = bacc.Bacc("TRN2", target_bir_lowering=False, debug=False)

    xq = nc.dram_tensor("xqT", [D, L], X_DT, kind="ExternalInput")
    xk = nc.dram_tensor("xkT", [D, L], X_DT, kind="ExternalInput")
    xv = nc.dram_tensor("xvT", [D, L], X_DT, kind="ExternalInput")
    wq = nc.dram_tensor("wqT", [D, EPC], X_DT, kind="ExternalInput")
    wk = nc.dram_tensor("wkT", [D, EPC], X_DT, kind="ExternalInput")
    wv = nc.dram_tensor("wvT", [D, EPC], X_DT, kind="ExternalInput")
    wo = nc.dram_tensor("woT", [EPC, D], F32, kind="ExternalInput")
    bqk = nc.dram_tensor("bqk", [2, EC, P], F32, kind="ExternalInput")
    ramps = nc.dram_tensor("ramps", [2, P], F32, kind="ExternalInput")
    out = nc.dram_tensor("outT", [D, L], BF16, kind="ExternalOutput")

    with tile.TileContext(nc) as tc:
        with (
            tc.tile_pool(name="consts", bufs=1) as consts,
            tc.tile_pool(name="acts", bufs=1) as acts,
        ):
            # ---- constants / weights
            wq_sb = consts.tile([P, NDC, EPC], X_DT, tag="wq")
            wk_sb = consts.tile([P, NDC, EPC], X_DT, tag="wk")
            wv_sb = consts.tile([P, NDC, EPC], X_DT, tag="wv")
            wv_r = wv.rearrange("(dc p) e -> p dc e", p=P)
            nc.sync.dma_start(wv_sb[:, 0:4, :], wv_r[:, 0:4, :])
            wo_sb = consts.tile([P, EC, D], F32R, tag="wo")
            wo_raw = consts.tile([P, EC, D], F32, tag="wo_raw")
            bqk_sb = consts.tile([P, 2, EC], F32, tag="bqk")
            ramps_sb = consts.tile([P, 2], F32, tag="ramps")

            ones_col = consts.tile([P, 1], F32, tag="ones_col")
            nc.vector.memset(ones_col[:], 1.0)
            c0i = consts.tile([P, NQ], mybir.dt.int32, tag="c0i")
            nc.gpsimd.iota(c0i[:], pattern=[[1, NQ]], base=0, channel_multiplier=0)
            c0h = consts.tile([P, NQ], F32, tag="c0h")
            nc.vector.tensor_copy(c0h[:], c0i[:])

            # ---- persistent activations
            qT = [acts.tile([P, L], F32R, tag=f"qT{c}", name=f"qT{c}") for c in range(EC)]
            kT = [acts.tile([P, L], F32R, tag=f"kT{c}", name=f"kT{c}") for c in range(EC)]
            aT = [acts.tile([P, L], F32R, tag=f"aT{c}", name=f"aT{c}") for c in range(EC)]
            vpk = [acts.tile([P, HPC, 66], F32R, tag=f"v{sb}", name=f"v{sb}") for sb in range(NSB)]

            # ================= phase 1: projections =================
            with (
                tc.tile_pool(name="xs", bufs=4) as xpool,
                tc.tile_pool(name="pjp", bufs=4, space=bass.MemorySpace.PSUM) as pjp,
            ):
                # v projection first (attention needs all of vpk; also the
                # heaviest DMA). Quad-grouped x tiles: one DMA feeds 4 s-blocks.
                xv_r = xv.rearrange("(dc p) l -> p dc l", p=P)
                for qd in range(NSB // 4 - 1, -1, -1):
                    xt = xpool.tile([P, NDC, 4 * P], X_DT, tag="xv", name="xv", bufs=3)
                    if qd == NSB // 4 - 1:
                        for dc in range(NDC):
                            nc.sync.dma_start(
                                xt[:, dc, :],
                                xv_r[:, dc, qd * 4 * P : (qd + 1) * 4 * P],
                            )
                            if dc == 3:
                                nc.sync.dma_start(wv_sb[:, 4:8, :], wv_r[:, 4:8, :])
                    elif qd == NSB // 4 - 2:
                        # halves in compute order (si descending) so the PE
                        # can start on the second pair while the first streams
                        nc.sync.dma_start(
                            xt[:, :, 2 * P : 4 * P],
                            xv_r[:, :, qd * 4 * P + 2 * P : (qd + 1) * 4 * P],
                        )
                        nc.sync.dma_start(
                            xt[:, :, 0 : 2 * P],
                            xv_r[:, :, qd * 4 * P : qd * 4 * P + 2 * P],
                        )
                    else:
                        nc.sync.dma_start(
                            xt[:], xv_r[:, :, qd * 4 * P : (qd + 1) * 4 * P]
                        )
                    for si in range(3, -1, -1):
                        sb = qd * 4 + si
                        psv = pjp.tile([P, EPC], F32, tag="vps", name="vps", bufs=2)
                        for dc in range(NDC):
                            nc.tensor.matmul(
                                psv[:],
                                xt[:, dc, si * P : (si + 1) * P],
                                wv_sb[:, dc, :],
                                start=(dc == 0),
                                stop=(dc == NDC - 1),
                            )
                        nc.scalar.copy(
                            vpk[sb][:, :, 0:64],
                            psv[:].rearrange("p (h e) -> p h e", e=64),
                        )
                        nc.vector.tensor_copy(
                            vpk[sb][:, :, 64:65],
                            ones_col[:, None, :].broadcast_to([P, HPC, 1]),
                        )

                nc.sync.dma_start(bqk_sb[:], bqk.rearrange("t c p -> p t c"))
                nc.sync.dma_start(ramps_sb[:], ramps.rearrange("t p -> p t"))

                for ti, (xdram, wsb) in enumerate(((xk, wk_sb), (xq, wq_sb))):
                    dst = (kT, qT)[ti]
                    bias_t = 1 - ti  # bqk row 0 = bq, row 1 = bk
                    wdram = (wk, wq)[ti]
                    nc.sync.dma_start(wsb[:], wdram.rearrange("(dc p) e -> p dc e", p=P))
                    for lp in ((1, 0) if ti == 0 else (0, 1)):  # k: tail first
                        ps = [
                            [pjp.tile([P, NQ], F32, tag="pj", name="pj", bufs=6) for _ in range(2)]
                            for _ in range(EC)
                        ]
                        for dc in range(NDC):
                            xt = xpool.tile([P, 2 * NQ], X_DT, tag=f"x{ti}", name="xt", bufs=8)
                            nc.sync.dma_start(
                                xt[:],
                                xdram[
                                    dc * P : (dc + 1) * P,
                                    lp * 2 * NQ : (lp + 1) * 2 * NQ,
                                ],
                            )
                            for ec in range(EC):
                                for l2 in range(2):
                                    nc.tensor.matmul(
                                        ps[ec][l2][:],
                                        wsb[:, dc, ec * P : (ec + 1) * P],
                                        xt[:, l2 * NQ : (l2 + 1) * NQ],
                                        start=(dc == 0),
                                        stop=(dc == NDC - 1),
                                    )
                        for l2 in range(2):
                            for ec in range(EC):
                                lc = lp * 2 + l2
                                # split each psum pair across Act/DVE so the
                                # last adds (gating the pool transition) drain
                                # in parallel; q's first chunk fully on Act
                                on_act = ec == 0 or (ti == 1 and lp == 0 and l2 == 0)
                                if on_act:
                                    nc.scalar.activation(
                                        dst[ec][:, lc * NQ : (lc + 1) * NQ],
                                        ps[ec][l2][:],
                                        mybir.ActivationFunctionType.Identity,
                                        bias=bqk_sb[:, bias_t, ec : ec + 1],
                                        scale=1.0,
                                    )
                                else:
                                    nc.vector.tensor_scalar_add(
                                        dst[ec][:, lc * NQ : (lc + 1) * NQ],
                                        ps[ec][l2][:],
                                        bqk_sb[:, bias_t, ec : ec + 1],
                                    )

                nc.sync.dma_start(wo_raw[:], wo.rearrange("(c p) n -> p c n", p=P))
                nc.gpsimd.tensor_copy(wo_sb[:], wo_raw[:])

            # ================= phase 2: attention =================
            with (
                tc.tile_pool(name="scp", bufs=2, space=bass.MemorySpace.PSUM) as scp,
                tc.tile_pool(name="avp", bufs=4, space=bass.MemorySpace.PSUM) as avp,
                tc.tile_pool(name="pp", bufs=7) as pp,
                tc.tile_pool(name="tp", bufs=4) as tp,
                tc.tile_pool(name="rdp", bufs=4) as rdp,
                tc.tile_pool(name="rbp", bufs=4) as rbp,
                tc.tile_pool(name="osb", bufs=4) as osb,
            ):
                def emit_op_group(jo, np2, st_on_act=False):
                    # out-projection columns [np2*256, np2*256+256) for q-chunk jo
                    qo = jo * NQ
                    ops = scp.tile([P, 2, NQ], F32, tag="sc", name="op")
                    for i in range(2):
                        n = np2 * 2 + i
                        for c in range(EC):
                            nc.tensor.matmul(
                                ops[:, i, :],
                                r(wo_sb[:, c, n * P : (n + 1) * P]),
                                r(aT[c][:, qo : qo + NQ]),
                                start=(c == 0),
                                stop=(c == EC - 1),
                            )
                    st = osb.tile([P, 2, NQ], BF16, tag="ost", name="ost")
                    if st_on_act:
                        nc.scalar.copy(st[:], ops[:])
                    else:
                        nc.vector.tensor_copy(st[:], ops[:])
                    nc.sync.dma_start(
                        out[np2 * 2 * P : (np2 + 1) * 2 * P, qo : qo + NQ].rearrange(
                            "(i p) q -> p i q", i=2
                        ),
                        st[:],
                    )

                for j in range(NJ):
                    q0 = j * NQ
                    js_lo = 4 * j
                    av = [avp.tile([65, NQ], F32, tag="av", name="av") for _ in range(HPC)]
                    # js descending: accumulation group starts at js=15 with a
                    # full-width MM; diagonal tiles then accumulate a prefix.
                    for js in range(NSB - 1, js_lo - 1, -1):
                        s0 = js * P
                        k = js - js_lo
                        masked = k <= 4
                        # visible q-prefix (min 256: f32r matmuls below N=256 run at 1/4 rate)
                        ev = min(NQ, max(2 * P, (k + 1) * P))
                        # exp only the truly visible prefix; for k=0 the mask
                        # zeroes cols 128:256 (stale-but-finite pool content)
                        ew = min(ev, (k + 1) * P)
                        tt = None
                        for pr in range(2):  # head pairs (0,1), (2,3)
                            sp = scp.tile([P, 2, NQ], F32, tag="sc", name="sc")
                            for i in range(2):
                                nc.tensor.matmul(
                                    sp[:, i, 0:ev],
                                    r(kT[pr][i * 64 : i * 64 + 64, s0 : s0 + P]),
                                    r(qT[pr][i * 64 : i * 64 + 64, q0 : q0 + ev]),
                                    start=True,
                                    stop=True,
                                )
                            pt = pp.tile([P, 2, NQ], F32R, tag="p", name="p")
                            nc.scalar.activation(
                                pt[:, :, 0:ew],
                                sp[:, :, 0:ew],
                                mybir.ActivationFunctionType.Exp,
                                scale=SCALE,
                            )
                            if masked:
                                if tt is None:
                                    tt = tp.tile([P, 1], F32, tag="t", name="t")
                                    ramp = ramps_sb[:, 1:2] if js == NSB - 1 else ramps_sb[:, 0:1]
                                    nc.vector.tensor_scalar_add(
                                        tt[:], ramp, float(s0 - q0)
                                    )
                                m0 = max(0, k * P - 8)
                                nc.vector.scalar_tensor_tensor(
                                    pt[:, :, m0:ev],
                                    c0h[:, None, m0:ev].broadcast_to([P, 2, ev - m0]),
                                    tt[:],
                                    pt[:, :, m0:ev],
                                    op0=mybir.AluOpType.is_le,
                                    op1=mybir.AluOpType.mult,
                                )
                            for i in range(2):
                                h = 2 * pr + i
                                nc.tensor.matmul(
                                    av[h][:, 0:ev],
                                    r(vpk[js][:, h, 0:65]),
                                    r(pt[:, i, 0:ev]),
                                    start=(js == NSB - 1),
                                    stop=(js == js_lo),
                                )
                    # out projection of the previous j-chunk, emitted
                    # BEFORE this j's normalize: its DVE st copies drain ahead
                    # of the normalize chain so score-psum slots free early.
                    # The last j normalizes first (no next-j work hides it) and
                    # its predecessor's sts go to the idle Act engine instead.
                    if 0 < j < NJ - 1:
                        for np2 in range(4 if j == 1 else 3):
                            emit_op_group(j - 1, np2, st_on_act=True)
                    if j == NJ - 1:
                        for np2 in range(4):
                            emit_op_group(j - 1, np2, st_on_act=True)
                        # deferred ops(1) group: PE filler while normalize(3)
                        # drains on DVE/Pool (nothing else remains at the tail)
                        for np2 in (3,):
                            emit_op_group(1, np2, st_on_act=True)

                    # all recips first: DVE runs them back-to-back while
                    # the Pool broadcasts pipeline behind them; muls last
                    rds, rbs = [], []
                    for h in range(HPC):
                        rd = rdp.tile([1, NQ], F32, tag="rd", bufs=4)
                        nc.vector.reciprocal(rd[:], av[h][64:65, :])
                        rds.append(rd)
                    for h in range(HPC):
                        rb = rbp.tile([64, NQ], F32, tag="rb", bufs=4)
                        nc.gpsimd.partition_broadcast(rb[:], rds[h][:], channels=64)
                        rbs.append(rb)
                    for h in range(HPC):
                        ec, r0 = h // 2, (h % 2) * 64
                        nc.vector.tensor_mul(
                            aT[ec][r0 : r0 + 64, q0 : q0 + NQ],
                            av[h][0:64, :],
                            rbs[h][:],
                        )
                # final j: fused st tile, quarter DMAs fire per copy
                stf = osb.tile([P, NDC, NQ], BF16, tag="stf", name="stf", bufs=1)
                qo = (NJ - 1) * NQ
                for np2 in range(4):
                    ops = scp.tile([P, 2, NQ], F32, tag="sc", name="op")
                    for i in range(2):
                        n = np2 * 2 + i
                        for c in range(EC):
                            nc.tensor.matmul(
                                ops[:, i, :],
                                r(wo_sb[:, c, n * P : (n + 1) * P]),
                                r(aT[c][:, qo : qo + NQ]),
                                start=(c == 0),
                                stop=(c == EC - 1),
                            )
                    if np2 % 2 == 1:
                        nc.scalar.copy(stf[:, np2 * 2 : np2 * 2 + 2, :], ops[:])
                    else:
                        nc.vector.tensor_copy(stf[:, np2 * 2 : np2 * 2 + 2, :], ops[:])
                    nc.sync.dma_start(
                        out[np2 * 2 * P : (np2 + 1) * 2 * P, qo : qo + NQ].rearrange(
                            "(i p) q -> p i q", i=2
                        ),
                        stf[:, np2 * 2 : np2 * 2 + 2, :],
                    )

    nc.compile()
    return nc


def make_in_maps(queries, keys, values, Wq, bq, Wk, bk, Wv, bv, Wo, bo):
    """Build per-core input maps. core = b*4 + g."""
    f32 = np.float32
    import ml_dtypes
    x_dt = ml_dtypes.bfloat16
    in_maps = []
    for core in range(8):
        b, g = core // 4, core % 4
        cols = slice(g * EPC, (g + 1) * EPC)
        off = 2 ** g
        ramp = (np.arange(P) - off).astype(f32)
        ramp_last = ramp.copy()
        ramp_last[P - 1] = 1e9  # s == L-1 always visible
        in_maps.append(
            {
                "xqT": np.ascontiguousarray(queries[b].T).astype(x_dt),
                "xkT": np.ascontiguousarray(keys[b].T).astype(x_dt),
                "xvT": np.ascontiguousarray(values[b].T).astype(x_dt),
                "wqT": np.ascontiguousarray(Wq[cols, :].T).astype(x_dt),
                "wkT": np.ascontiguousarray(Wk[cols, :].T).astype(x_dt),
                "wvT": np.ascontiguousarray(Wv[cols, :].T).astype(x_dt),
                "woT": np.ascontiguousarray(Wo[:, cols].T, dtype=f32),
                "bqk": np.stack(
                    [bq[cols].reshape(EC, P), bk[cols].reshape(EC, P)]
                ).astype(f32),
                "ramps": np.stack([ramp, ramp_last]),
            }
        )
    return in_maps


def gather_outputs(results, Wo, bv, bo):
    """results: list of 8 dicts with 'outT' [D, L] bf16. Returns [2, L, D] f32."""
    host_bias = (Wo.astype(np.float64) @ bv.astype(np.float64) + bo).astype(
        np.float32
    )
    out = np.zeros((2, L, D), np.float32)
    for b in range(2):
        acc = np.zeros((D, L), np.float32)
        for g in range(4):
            acc += np.asarray(results[b * 4 + g]["outT"]).astype(np.float32)
        out[b] = acc.T + host_bias[None, :]
    return out




# ======================= host entry point =======================
_NC_CACHE = None


def kernel(queries, keys, values, Wq, bq, Wk, bk, Wv, bv, Wo, bo):
    """Full-input entry: shards across 8 NeuronCores, returns [2, 2048, 1024]."""
    global _NC_CACHE
    from concourse.bass_utils import run_bass_kernel_spmd

    args = [np.asarray(a) for a in (queries, keys, values, Wq, bq, Wk, bk, Wv, bv, Wo, bo)]
    queries, keys, values, Wq, bq, Wk, bk, Wv, bv, Wo, bo = args
    if _NC_CACHE is None:
        _NC_CACHE = build_nc()
    in_maps = make_in_maps(queries, keys, values, Wq, bq, Wk, bk, Wv, bv, Wo, bo)
    res = run_bass_kernel_spmd(_NC_CACHE, in_maps, list(range(8)))
    return gather_outputs(res.results, Wo, bv, bo)
